# revision 5
# baseline (speedup 1.0000x reference)
"""Trainium2 Bass kernel for DynamicResidualStageWrapper (18-block MLP stage
with channel-gated anchor routing), data-parallel over batch across 8 cores.

Contract: kernel(**inputs) takes FULL unsharded inputs (numpy, keyed as in
reference.setup_inputs()) and returns the FULL output [32,14,14,512].

Per-core layout: activations transposed as [C=512, T=784] in 4 k-tiles
[128, 784] bf16; tokens are (sample b, position hw); everything bf16
(tolerance 2e-2, measured ~5e-3).

Structure (v2, rewritten from the chunked baseline):
- normal blocks: ct-major matmuls (FD=392 per pair) into 2-bank psum tiles;
  ONE gelu per ct at FD=784 (halves ACT instruction count; ACT has a fixed
  ~300ns per-instruction cost). The last ct's gelu is split per-pair so the
  next block's k=ct3 matmuls aren't gated on a full-width gelu.
- routers (11,14,17): pair-major. Pooling uses linearity of the mean:
  z = fc1_w.T @ Xn on the PE (4 small matmuls) + one DVE reduce gives
  pooled@fc1 directly -- no per-sample accumulation pass. The MLP + softmax
  run per sample-pair so pair 0's corrections (and the next block's pair-0
  matmuls) start while pair 1 is still in flight. Corrections split between
  DVE (scalar_tensor_tensor) and ACT (Identity with per-partition scale).
- block 17 streams output pieces per (sample, k) over all three DMA queues.
- startup: contiguous host-side layouts (x pieces, weight ct-halves) so the
  first block's operands arrive ~3us earlier; HAM warm-up dummies run only
  during the startup DMA window.
"""

import numpy as np

import concourse.bacc as bacc
import concourse.mybir as mybir
import concourse.tile as tile
from concourse.bass_utils import run_bass_kernel_spmd

# ---- problem constants (hardcoded per spec) ----
NUM_BLOCKS = 18
ANCHOR_IDX = (1, 4, 9)
TARGET_IDX = (11, 14, 17)
POST_ROUTER = (12, 15)
C = 512
HID = 128
A = 3
B, H, W = 32, 14, 14
N_CORES = 8
BL = B // N_CORES          # 4 samples per core
HW = H * W                 # 196 positions per sample
T = BL * HW                # 784 tokens per core
KT = C // 128              # 4 channel tiles
NP = 2                     # sample pairs per core
PW = T // NP               # 392 tokens per pair
CHALF = KT * 256           # cols per ct-half weight piece

F32 = mybir.dt.float32
BF16 = mybir.dt.bfloat16
GELU = mybir.ActivationFunctionType.Gelu_apprx_tanh
TANH = mybir.ActivationFunctionType.Tanh
IDENT = mybir.ActivationFunctionType.Identity
MUL = mybir.AluOpType.mult
ADD = mybir.AluOpType.add
_cached = {}


def _w_ap(w_t, k, ct):
    """lhsT tile (k, ct) from a [128, 2, KT*256] weight tile."""
    off = k * 256 + (ct % 2) * 128
    return w_t[:, ct // 2, off:off + 128]


def build_program():
    nc = bacc.Bacc(trn_type="TRN2", target_bir_lowering=False, debug=False)

    xT = nc.dram_tensor("xT", [KT, NP, 128, PW], BF16, kind="ExternalInput").ap()
    wd = nc.dram_tensor("wd", [NUM_BLOCKS, 2, 128, CHALF], BF16,
                        kind="ExternalInput").ap()
    bias_cols = nc.dram_tensor("bias_cols", [128, NUM_BLOCKS * KT], F32,
                               kind="ExternalInput").ap()
    fc1w = nc.dram_tensor("fc1w", [128, A * KT * 128], BF16, kind="ExternalInput").ap()
    fc1b = nc.dram_tensor("fc1b", [128, A], F32, kind="ExternalInput").ap()
    fc2w = nc.dram_tensor("fc2w", [128, A * A * C], BF16, kind="ExternalInput").ap()
    fc2bias = nc.dram_tensor("fc2bias", [128, A * A * KT * BL], F32,
                             kind="ExternalInput").ap()
    gbc = nc.dram_tensor("gbc", [128, 2 * A], BF16, kind="ExternalInput").ap()
    gbc32 = nc.dram_tensor("gbc32", [128, A], F32, kind="ExternalInput").ap()
    outT = nc.dram_tensor("outT", [KT, NP, 128, PW], BF16, kind="ExternalOutput").ap()

    anchor_of = {b_: i for i, b_ in enumerate(ANCHOR_IDX)}
    target_of = {b_: i for i, b_ in enumerate(TARGET_IDX)}

    with tile.TileContext(nc) as tc:
        with (
            tc.tile_pool(name="const", bufs=1) as cpool,
            tc.tile_pool(name="wpool", bufs=6) as wpool,
            tc.tile_pool(name="xpool", bufs=3) as xpool,
            tc.tile_pool(name="apool", bufs=1) as apool,
            tc.tile_pool(name="rpool", bufs=2) as rpool,
            tc.tile_pool(name="ppool", bufs=3, space="PSUM") as ppool,
            tc.tile_pool(name="fcps", bufs=2, space="PSUM") as fcps,
        ):
            # ---- HAM warm-up scaffolding (PE busy during startup DMA wait)
            zw = cpool.tile([128, 128], BF16, name="zw")
            zx = cpool.tile([128, 512], BF16, name="zx")
            warm = cpool.tile([128, 1], F32, name="warm")
            nc.gpsimd.memset(warm[:], 0.0)
            nc.vector.memset(zw[:], 0.0)
            nc.vector.memset(zx[:], 0.0)

            # ---- startup DMAs.
            # sync:   x pair-0 pieces (k=0..3), then w1 halves
            # scalar: bias, ACT-table warms, then w2/w3 halves
            # gpsimd: w0 halves, x pair-1 pieces, small fc consts
            X = [xpool.tile([128, T], BF16, tag=f"x{k}", name=f"xin{k}")
                 for k in range(KT)]
            wtiles = {}
            for k in range(KT):
                nc.sync.dma_start(X[k][:, 0:PW], xT[k, 0])
            bias_t = cpool.tile([128, NUM_BLOCKS * KT], F32, name="bias_t")
            nc.scalar.dma_start(bias_t[:], bias_cols[:])
            # table loads for the whole gelu/tanh/identity set fire here,
            # before ACT is needed (first real gelu ~10us in)
            nc.scalar.activation(warm[:], warm[:], GELU)
            nc.scalar.activation(warm[:], warm[:], TANH)
            nc.scalar.activation(warm[:], warm[:], IDENT)
            w0 = wpool.tile([128, 2, CHALF], BF16, tag="w", name="w0")
            nc.gpsimd.dma_start(w0[:, 0, :], wd[0, 0])
            nc.gpsimd.dma_start(w0[:, 1, :], wd[0, 1])
            wtiles[0] = w0
            # dummy matmuls keep the PE HAM clock warming during the DMA wait
            _dummy_mms(nc, fcps, zw, zx, n512=8)
            for k in range(KT):
                nc.gpsimd.dma_start(X[k][:, PW:T], xT[k, 1])
            w1 = wpool.tile([128, 2, CHALF], BF16, tag="w", name="w1")
            nc.sync.dma_start(w1[:, 0, :], wd[1, 0])
            nc.sync.dma_start(w1[:, 1, :], wd[1, 1])
            wtiles[1] = w1
            for i in (2, 3):
                w_n = wpool.tile([128, 2, CHALF], BF16, tag="w", name=f"w{i}")
                nc.scalar.dma_start(w_n[:, 0, :], wd[i, 0])
                nc.scalar.dma_start(w_n[:, 1, :], wd[i, 1])
                wtiles[i] = w_n
            fc1b_t = cpool.tile([128, A], F32, name="fc1b_t")
            nc.gpsimd.dma_start(fc1b_t[:], fc1b[:])
            gbc_t = cpool.tile([128, 2 * A], BF16, name="gbc_t")
            nc.gpsimd.dma_start(gbc_t[:], gbc[:])
            gbc32_t = cpool.tile([128, A], F32, name="gbc32_t")
            nc.gpsimd.dma_start(gbc32_t[:], gbc32[:])
            fc1w_t, fc2w_t, fc2bias_t = {}, {}, {}

            anchors = {}
            ga2 = {}     # t -> [gamma_t * a2 per k]
            state = {"adiff": None}
            Xr = None    # corrected-output tiles for the upcoming router

            for i in range(NUM_BLOCKS):
                t_idx = target_of.get(i)
                a_idx = anchor_of.get(i)

                # prefetch block weights 4 blocks out (alternate sync/gpsimd)
                if i + 4 < NUM_BLOCKS and (i + 4) not in wtiles:
                    w_n = wpool.tile([128, 2, CHALF], BF16, tag="w",
                                     name=f"w{i + 4}")
                    eng = nc.sync if i % 2 == 0 else nc.gpsimd
                    eng.dma_start(w_n[:, 0, :], wd[i + 4, 0])
                    eng.dma_start(w_n[:, 1, :], wd[i + 4, 1])
                    wtiles[i + 4] = w_n
                # prefetch router weights ~3 blocks out
                if i + 3 in target_of:
                    tt = target_of[i + 3]
                    f1 = cpool.tile([128, KT * 128], BF16, name=f"fc1w_{tt}")
                    nc.gpsimd.dma_start(
                        f1[:], fc1w[:, tt * KT * 128:(tt + 1) * KT * 128])
                    fc1w_t[tt] = f1
                    f2 = cpool.tile([128, A * C], BF16, name=f"fc2w_{tt}")
                    nc.gpsimd.dma_start(
                        f2[:], fc2w[:, tt * A * C:(tt + 1) * A * C])
                    fc2w_t[tt] = f2
                    fb = cpool.tile([128, A * KT * BL], F32, name=f"fc2b_{tt}")
                    nc.gpsimd.dma_start(
                        fb[:], fc2bias[:, tt * A * KT * BL:(tt + 1) * A * KT * BL])
                    fc2bias_t[tt] = fb

                w_t = wtiles.pop(i)

                Xn = []
                for k in range(KT):
                    if a_idx is not None:
                        xn = apool.tile([128, T], BF16, tag=f"a{a_idx}_{k}",
                                        name=f"anc{a_idx}_{k}")
                    else:
                        xn = xpool.tile([128, T], BF16, tag=f"x{k}",
                                        name=f"xb{i}_{k}")
                    Xn.append(xn)

                pair_major = (i == 0 or t_idx is not None or i in POST_ROUTER)
                if pair_major:
                    # p outer so pair 0 completes (mm+gelu) before pair 1;
                    # routers hang their per-pair MLP off this, and block 0
                    # matches the startup DMA arrival order.
                    for p in range(NP):
                        for cth in range(2):
                            ps = ppool.tile([128, 1024], F32, tag="mm",
                                            name=f"ps{i}_{p}_{cth}")
                            for ct in (2 * cth, 2 * cth + 1):
                                sl = slice((ct % 2) * 512, (ct % 2) * 512 + PW)
                                for k in range(KT):
                                    nc.tensor.matmul(
                                        ps[:, sl], _w_ap(w_t, k, ct),
                                        X[k][:, p * PW:(p + 1) * PW],
                                        start=(k == 0), stop=(k == KT - 1))
                            for ct in (2 * cth, 2 * cth + 1):
                                sl = slice((ct % 2) * 512, (ct % 2) * 512 + PW)
                                nc.scalar.activation(
                                    Xn[ct][:, p * PW:(p + 1) * PW], ps[:, sl],
                                    GELU,
                                    bias=bias_t[:, i * KT + ct:i * KT + ct + 1])
                        if t_idx is not None:
                            _router_pair(nc, rpool, fcps, t_idx, p, Xn,
                                         anchors, ga2, state["adiff"],
                                         fc1w_t, fc1b_t, fc2w_t, fc2bias_t,
                                         gbc32_t, Xr,
                                         outT if i == NUM_BLOCKS - 1 else None)
                else:
                    # ct-major; one FD=784 gelu per ct (last ct split per-pair
                    # so the next block's k=3 matmuls aren't over-gated)
                    for ct in range(KT):
                        ps = ppool.tile([128, 1024], F32, tag="mm",
                                        name=f"ps{i}_{ct}")
                        for p in range(NP):
                            sl = slice(p * 512, p * 512 + PW)
                            for k in range(KT):
                                nc.tensor.matmul(
                                    ps[:, sl], _w_ap(w_t, k, ct),
                                    X[k][:, p * PW:(p + 1) * PW],
                                    start=(k == 0), stop=(k == KT - 1))
                        bias_ap = bias_t[:, i * KT + ct:i * KT + ct + 1]
                        if ct < KT - 1:
                            nc.scalar.activation(
                                Xn[ct][:].rearrange("q (c x) -> q c x", c=NP),
                                ps[:].rearrange("q (c x) -> q c x", c=NP)[:, :, 0:PW],
                                GELU, bias=bias_ap)
                        else:
                            for p in range(NP):
                                nc.scalar.activation(
                                    Xn[ct][:, p * PW:(p + 1) * PW],
                                    ps[:, p * 512:p * 512 + PW], GELU,
                                    bias=bias_ap)

                if a_idx is not None:
                    anchors[a_idx] = Xn
                    if a_idx == 2:
                        # gamma_t * a2 (gate-independent part of the update),
                        # then anchor diffs in place over a0/a1 (dead values):
                        # routed = gamma*a2 + g0*(a0-a2) + g1*(a1-a2)
                        for t in range(A):
                            ga2[t] = []
                            for k in range(KT):
                                g2 = apool.tile([128, T], BF16, tag=f"ga{t}_{k}",
                                                name=f"ga{t}_{k}")
                                nc.vector.tensor_scalar_mul(
                                    g2[:], Xn[k][:], gbc32_t[:, t:t + 1])
                                ga2[t].append(g2)
                        adiff = {}
                        for da in range(2):
                            adiff[da] = []
                            for k in range(KT):
                                dt_ = anchors[da][k]
                                nc.vector.tensor_sub(dt_[:], dt_[:],
                                                     anchors[2][k][:])
                                adiff[da].append(dt_)
                        state["adiff"] = adiff

                if t_idx is not None:
                    X = Xr   # corrected activations written by _router_pair
                else:
                    X = Xn
                # allocate Xr for an upcoming router block
                if (i + 1) in target_of:
                    Xr = [xpool.tile([128, T], BF16, tag=f"x{k}",
                                     name=f"xr{target_of[i + 1]}_{k}")
                          for k in range(KT)]

    nc.compile()
    return nc


_dummy_ctr = [0]


def _dummy_mms(nc, pool, zw, zx, n512):
    """Dependency-free matmuls on zeroed tiles keep the PE HAM clock warm
    across the startup DMA window."""
    _dummy_ctr[0] += 1
    ps = pool.tile([128, 512], F32, tag="fc", name=f"dummy{_dummy_ctr[0]}")
    for _ in range(n512):
        nc.tensor.matmul(ps[:, 0:512], zw[:], zx[:], start=True, stop=True)


def _router_pair(nc, rpool, fcps, t, p, Xn, anchors, ga2, adiff,
                 fc1w_t, fc1b_t, fc2w_t, fc2bias_t, gbc32_t, Xr, outT):
    """Per-pair ChannelGating router: z = fc1^T Xn (PE) -> hw-reduce ->
    gelu -> fc2 -> softmax over anchors -> per-sample corrections into Xr."""
    # z = fc1_w.T @ Xn  [HID, PW]; mean-pool folds into the hw reduce
    zps = fcps.tile([128, 512], F32, tag="fc", name=f"zps{t}_{p}")
    for k in range(KT):
        nc.tensor.matmul(zps[:, 0:PW], fc1w_t[t][:, k * 128:(k + 1) * 128],
                         Xn[k][:, p * PW:(p + 1) * PW],
                         start=(k == 0), stop=(k == KT - 1))
    # base term of the update: xr = Xn + gamma*a2 (gate-independent)
    for k in range(KT):
        sl = slice(p * PW, (p + 1) * PW)
        nc.vector.tensor_add(Xr[k][:, sl], Xn[k][:, sl], ga2[t][k][:, sl])
    zred = rpool.tile([128, NP], F32, tag="zred", name=f"zred{t}_{p}")
    nc.vector.tensor_reduce(
        zred[:], zps[:, 0:PW].rearrange("q (b x) -> q b x", b=NP),
        axis=mybir.AxisListType.X, op=ADD)
    h = rpool.tile([128, NP], BF16, tag="h", name=f"h{t}_{p}")
    nc.scalar.activation(h[:], zred[:], GELU, bias=fc1b_t[:, t:t + 1])

    # fc2: logits col-tiles [128, (a, k, b)]  (12 x 2 cols)
    NJ = A * KT
    ps2 = fcps.tile([128, 512], F32, tag="fc", name=f"ps2{t}_{p}")
    for j in range(NJ):
        nc.tensor.matmul(ps2[:, j * NP:(j + 1) * NP],
                         fc2w_t[t][:, j * 128:(j + 1) * 128],
                         h[:], start=True, stop=True)
    logits = rpool.tile([128, NJ * NP], F32, tag="lg", name=f"lg{t}_{p}")
    nc.vector.tensor_add(
        logits[:].rearrange("q (j b) -> q j b", b=NP),
        ps2[:, 0:NJ * NP].rearrange("q (j b) -> q j b", b=NP),
        fc2bias_t[t][:].rearrange("q (j b) -> q j b", b=BL)[:, :, 2 * p:2 * p + 2])

    # softmax over a (cols = a*8 + k*2 + bb); exp via tanh identity
    KB = KT * NP  # 8
    th = rpool.tile([128, A * KB], F32, tag="th", name=f"th{t}_{p}")
    nc.scalar.activation(th[:], logits[:], TANH, scale=0.5)
    den = rpool.tile([128, A * KB], F32, tag="den", name=f"den{t}_{p}")
    nc.vector.tensor_scalar(den[:], th[:], -1.0, 1.0, op0=MUL, op1=ADD)
    rec = rpool.tile([128, A * KB], F32, tag="rec", name=f"rec{t}_{p}")
    nc.vector.reciprocal(rec[:], den[:])
    e = rpool.tile([128, A * KB], F32, tag="e", name=f"e{t}_{p}")
    nc.vector.tensor_scalar(e[:], rec[:], 2.0, -1.0, op0=MUL, op1=ADD)
    s = rpool.tile([128, KB], F32, tag="s", name=f"s{t}_{p}")
    nc.vector.tensor_reduce(s[:], e[:].rearrange("q (a m) -> q m a", a=A),
                            axis=mybir.AxisListType.X, op=ADD)
    rinv = rpool.tile([128, KB], F32, tag="rinv", name=f"rinv{t}_{p}")
    nc.vector.reciprocal(rinv[:], s[:])
    rg = rpool.tile([128, KB], F32, tag="rg", name=f"rg{t}_{p}")
    nc.vector.tensor_scalar_mul(rg[:], rinv[:], gbc32_t[:, t:t + 1])
    g = rpool.tile([128, 2 * KB], BF16, tag="g", name=f"g{t}_{p}")
    g32 = rpool.tile([128, 2 * KB], F32, tag="g32", name=f"g32{t}_{p}")
    with nc.allow_low_precision(reason="gates round to bf16 on write"):
        for a in range(2):
            nc.vector.tensor_mul(g32[:, a * KB:(a + 1) * KB],
                                 e[:, a * KB:(a + 1) * KB], rg[:])
            nc.vector.tensor_mul(g[:, a * KB:(a + 1) * KB],
                                 e[:, a * KB:(a + 1) * KB], rg[:])

    # per-sample corrections: xr += g0*(a0-a2) + g1*(a1-a2).
    # anchor 0 via DVE STT; anchor 1 via ACT Identity (per-partition scale)
    # + a 2x-mode DVE add, so the two engines run in parallel.
    dq = [nc.sync, nc.gpsimd, nc.scalar]
    for bb in range(NP):
        b = 2 * p + bb
        sl = slice(b * HW, (b + 1) * HW)
        for k in range(KT):
            c0 = 0 * KB + k * NP + bb
            c1 = 1 * KB + k * NP + bb
            nc.vector.scalar_tensor_tensor(
                Xr[k][:, sl], adiff[0][k][:, sl], g[:, c0:c0 + 1],
                Xr[k][:, sl], op0=MUL, op1=ADD)
            tmp = rpool.tile([128, HW], BF16, tag=f"tmp{k}",
                             name=f"tmp{t}_{b}_{k}")
            nc.scalar.activation(tmp[:], adiff[1][k][:, sl], IDENT,
                                 scale=g32[:, c1:c1 + 1])
            nc.vector.tensor_add(Xr[k][:, sl], Xr[k][:, sl], tmp[:])
            if outT is not None:
                eng = dq[(b * KT + k) % 3]
                eng.dma_start(outT[k, p, :, bb * HW:(bb + 1) * HW],
                              Xr[k][:, sl])


def _prep_shared(block_w, block_b, fc1_w, fc1_b, fc2_w, fc2_b, gammas):
    """Host-side packing of the (replicated) weight tensors."""
    import ml_dtypes
    f = np.float32
    bf = ml_dtypes.bfloat16
    w = np.asarray(block_w, dtype=f)
    # [NB, 2(ct-half), 128(p=cin within k), KT*256] -- each half contiguous
    wd = np.ascontiguousarray(
        w.reshape(NUM_BLOCKS, KT, 128, 2, 256).transpose(0, 3, 2, 1, 4)
        .reshape(NUM_BLOCKS, 2, 128, CHALF).astype(bf))
    bias_cols = np.ascontiguousarray(
        np.asarray(block_b, dtype=f).reshape(NUM_BLOCKS * KT, 128).T, dtype=f)
    fc1s = (np.asarray(fc1_w, dtype=f) / float(HW)).astype(f)   # [A, C, HID]
    fc1w_cat = np.concatenate(
        [fc1s[t][k * 128:(k + 1) * 128, :] for t in range(A) for k in range(KT)],
        axis=1)                                               # [128, A*KT*128]
    fc1b_cols = np.ascontiguousarray(np.asarray(fc1_b, dtype=f).T)  # [128, A]
    fc2w_cat = np.concatenate([np.asarray(fc2_w[t], dtype=f) for t in range(A)],
                              axis=1)                          # [128, A*A*C]
    fc2bias = np.concatenate(
        [np.repeat(np.asarray(fc2_b[t], dtype=f).reshape(A * KT, 128).T,
                   BL, axis=1) for t in range(A)], axis=1)     # [128, A*A*KT*BL]
    gam = np.asarray(gammas, dtype=f)
    gbc = np.broadcast_to(np.concatenate([gam, -gam])[None, :], (128, 2 * A))
    gbc = np.ascontiguousarray(gbc.astype(bf))
    gbc32 = np.ascontiguousarray(np.broadcast_to(gam[None, :], (128, A)))
    return dict(wd=wd, bias_cols=bias_cols, gbc32=gbc32,
                fc1w=np.ascontiguousarray(fc1w_cat.astype(bf)),
                fc1b=fc1b_cols,
                fc2w=np.ascontiguousarray(fc2w_cat.astype(bf)),
                fc2bias=np.ascontiguousarray(fc2bias), gbc=gbc)


def shard_x(x):
    """Full x [B,H,W,C] -> per-core [KT, NP, 128, PW] bf16 (contig pieces)."""
    import ml_dtypes
    shards = []
    for r in range(N_CORES):
        xs = np.asarray(x[r * BL:(r + 1) * BL], dtype=np.float32)  # [BL,H,W,C]
        xt = xs.reshape(T, C).T                                    # [C, T]
        shards.append(np.ascontiguousarray(
            xt.reshape(KT, 128, NP, PW).transpose(0, 2, 1, 3)
            .astype(ml_dtypes.bfloat16)))
    return shards


def unshard_out(outs):
    """Per-core [KT, NP, 128, PW] results -> full [B,H,W,C]."""
    parts = []
    for o in outs:
        ot = np.asarray(o, dtype=np.float32).transpose(0, 2, 1, 3)  # k,128,p,pw
        parts.append(ot.reshape(C, T).T.reshape(BL, H, W, C))
    return np.ascontiguousarray(np.concatenate(parts, axis=0), dtype=np.float32)


def kernel(x, block_w, block_b, fc1_w, fc1_b, fc2_w, fc2_b, gammas):
    if "nc" not in _cached:
        _cached["nc"] = build_program()
    nc = _cached["nc"]

    shared = _prep_shared(block_w, block_b, fc1_w, fc1_b, fc2_w, fc2_b, gammas)
    xs = shard_x(x)
    in_maps = [dict(shared, xT=xs[r]) for r in range(N_CORES)]
    res = run_bass_kernel_spmd(nc, in_maps, list(range(N_CORES)))
    return unshard_out([res.results[r]["outT"] for r in range(N_CORES)])
